# revision 1
# baseline (speedup 1.0000x reference)
"""Blocksparse conv2d (3x3, stride 1, pad 1) on 8 Trainium2 NeuronCores.

Strategy
--------
Data-parallel over batch: 16 images -> 2 per core, identical SPMD program.

The mask zeroes whole 32x32 (cout, cin) channel blocks. The schedule is
specialized at build time from the *runtime* mask values (host inspects the
numpy mask inside kernel(), so any mask is handled correctly):
  - input-channel blocks that are entirely masked out are never loaded,
  - when only K_used <= 64 input channels survive, the channels are
    replicated into D = 128//K_used partition row-groups so that D
    row-tiled matmuls (different PE row groups) run concurrently.

Conv is lowered to 9 shifted matmuls accumulating in PSUM over a flat
row-padded image in SBUF (one zero row above and below, NO column
padding, so the x load is a single fully contiguous DMA per replica —
column-padded layouts need one DMA descriptor per image row, and SWDGE
descriptor emission at ~20ns each dominated the whole kernel).  Without
column zeros, the dw=0/dw=2 taps wrap across row boundaries, corrupting
output columns 0 and 127; those two columns are recomputed exactly with
12 small column-strided matmuls per image and overwritten during
copy-out.

Inputs are rounded to float32r during the load DMA (free SWDGE cast),
giving full-rate PE streaming at ~1e-4 matmul error.  Bias is fused into
the PSUM->SBUF copy on the scalar engine.
"""

import numpy as np
from contextlib import ExitStack

import concourse.bass as bass
import concourse.tile as tile
from concourse import mybir, bacc
from concourse import bass_utils
from concourse.masks import make_identity

# Problem shape (hardcoded per contract)
B, CIN, COUT, H, W = 16, 128, 128, 128, 128
KH, KW = 3, 3
BLK = 32
NCORES = 8
BPC = B // NCORES            # images per core

IMG0 = 1 + W                 # flat offset of image row 0 (1 slack el + top pad row)
LFLAT = IMG0 + H * W + W + 1  # + bottom pad row + 1 slack el
RPW = 3                      # output rows per PSUM window
NWIN = (H + RPW - 1) // RPW  # 43 windows (42 full + one 2-row)
CHUNK_WIN = 8                # windows staged per output DMA (24 rows)
GROUP_WIN = 4                # windows per matmul group (tap-outer ordering)

_cache = {}


def _build(n_ib, D, mm_order="t_outer", repeat=1, mm_dtype="f32r", taps=9, x_mode="swdge_cast"):
    """Build + compile the per-core SPMD program.

    n_ib: number of surviving 32-channel input blocks (1..4)
    D:    partition replication factor (1, 2 or 4); D*K_used <= 128
    repeat/taps/mm_*: benchmarking knobs (repeat>1 wraps compute in For_i)
    """
    K_used = BLK * n_ib
    DK = D * K_used
    assert DK <= 128

    nc = bacc.Bacc("TRN2", target_bir_lowering=False, debug=False)
    f32 = mybir.dt.float32
    f32r = mybir.dt.float32r if mm_dtype == "f32r" else mybir.dt.bfloat16

    # channels arrive pre-replicated D times from the host (a step-0
    # replicated-source DMA measured ~7x below line rate)
    x_in = nc.dram_tensor("x", [BPC, DK, H, W], f32, kind="ExternalInput").ap()
    w_in = nc.dram_tensor("w", [COUT, K_used, KH, KW], f32, kind="ExternalInput").ap()
    m_in = nc.dram_tensor("m", [COUT, K_used, KH, KW], f32, kind="ExternalInput").ap()
    b_in = nc.dram_tensor("bias", [COUT], f32, kind="ExternalInput").ap()
    y_out = nc.dram_tensor("y", [BPC, COUT, H, W], f32, kind="ExternalOutput").ap()

    with tile.TileContext(nc) as tc:
        with ExitStack() as ctx:
            singles = ctx.enter_context(tc.tile_pool(name="singles", bufs=1))
            stage_pool = ctx.enter_context(tc.tile_pool(name="ystage", bufs=3))
            psum_pool = ctx.enter_context(
                tc.tile_pool(name="psum", bufs=6, space="PSUM")
            )
            epsum_pool = ctx.enter_context(
                tc.tile_pool(name="epsum", bufs=2, space="PSUM")
            )

            # ---- one-time setup: weights, bias, identity ------------------
            ident = singles.tile([128, 128], f32, name="ident")
            make_identity(nc, ident)

            bias_sb = singles.tile([COUT, 1], f32, name="bias_sb")
            nc.sync.dma_start(out=bias_sb, in_=b_in.unsqueeze(1))

            w_raw = singles.tile([COUT, K_used, KH * KW], f32, name="w_raw")
            m_raw = singles.tile([COUT, K_used, KH * KW], f32, name="m_raw")
            nc.sync.dma_start(out=w_raw, in_=w_in.rearrange("o c kh kw -> o c (kh kw)"))
            nc.sync.dma_start(out=m_raw, in_=m_in.rearrange("o c kh kw -> o c (kh kw)"))
            nc.vector.tensor_tensor(
                out=w_raw, in0=w_raw, in1=m_raw, op=mybir.AluOpType.mult
            )

            # wT[d*K_used + c, t, o] = (w*m)[o, c, tap t], replicated D times
            # along partitions so each PE row-group has its own copy.
            wm2 = singles.tile([COUT, 128], f32, name="wm2")
            wT = singles.tile([128, KH * KW, COUT], f32r, name="wT")
            for t in range(KH * KW):
                for s in range(D):
                    nc.vector.tensor_copy(
                        out=wm2[:, s * K_used : (s + 1) * K_used],
                        in_=w_raw[:, :, t],
                    )
                ps_t = psum_pool.tile([128, 512], f32, tag="ps", name=f"ps_t{t}")
                nc.tensor.transpose(ps_t[:DK, :COUT], wm2[:, :DK], ident)
                nc.vector.tensor_copy(out=wT[:DK, t, :], in_=ps_t[:DK, :COUT])

            # ---- persistent flat row-padded x buffers ---------------------
            # memset doesn't support float32r; pad zeros come from DVE copies
            # (fp32 -> f32r cast) out of a zeroed fp32 scratch.
            zsc = singles.tile([128, IMG0], f32, name="zsc")
            nc.gpsimd.memset(zsc, 0.0)
            xbufs = []
            for i in range(BPC):
                xb = singles.tile([128, LFLAT], f32r, name=f"xbuf{i}")
                nc.vector.tensor_copy(out=xb[:DK, :IMG0], in_=zsc[:DK, :])
                nc.vector.tensor_copy(
                    out=xb[:DK, IMG0 + H * W :], in_=zsc[:DK, : W + 1]
                )
                xbufs.append(xb)

            # ---- per-image pipeline --------------------------------------
            if repeat > 1:
                loop_cm = tc.For_i(0, repeat, 1)
                loop_cm.__enter__()
            for b in range(BPC):
                xb = xbufs[b]
                if taps < 0:
                    nc.vector.tensor_copy(out=xb[:DK, 0:8], in_=zsc[:DK, 0:8])
                    continue
                src = x_in[b].rearrange("c h w -> c (h w)")  # [DK, H*W]
                dst = xb[:DK, IMG0 : IMG0 + H * W]
                if x_mode == "swdge_cast":
                    # fp32 -> f32r rounding happens inside the SWDGE DMA
                    nc.gpsimd.dma_start(out=dst, in_=src)
                else:  # hwdge_f32: timing probe only (wrong dtype for MMs)
                    nc.sync.dma_start(out=dst.bitcast(mybir.dt.float32), in_=src)
                if taps == 0:
                    continue

                # -- exact edge columns (j=0 and j=127) into edge psum ------
                # main-path taps wrap across rows there; recompute from
                # column-strided views and overwrite during copy-out.
                edge_ps = epsum_pool.tile([128, 512], f32, tag="eps", name=f"eps{b}")
                # NOTE: keep each edge's 6 accumulating matmuls contiguous —
                # start=True resets the whole bank's has_written bits, so
                # interleaving the two groups corrupts the first one.
                edge_mms = []  # (psum_off, t, col)
                for dh in range(KH):
                    for dwsel in (1, 2):  # left edge j=0
                        edge_mms.append((0, dh * 3 + dwsel, dwsel - 1))
                for dh in range(KH):
                    for dwsel in (0, 1):  # right edge j=127
                        edge_mms.append((128, dh * 3 + dwsel, 126 + dwsel))
                n_per = {0: 0, 128: 0}
                for eoff, _, _ in edge_mms:
                    n_per[eoff] += 1
                seen = {0: 0, 128: 0}
                for eoff, t, col in edge_mms:
                    dh = t // 3
                    o = IMG0 + (dh - 1) * W + col
                    v = xb[0:K_used, o : o + 1]
                    rhs = bass.AP(
                        tensor=v.tensor,
                        offset=v.offset,
                        ap=[list(v.ap[0]), [W, H]],
                    )
                    nc.tensor.matmul(
                        edge_ps[:, eoff : eoff + H],
                        wT[0:K_used, t, :],
                        rhs,
                        start=(seen[eoff] == 0),
                        stop=(seen[eoff] == n_per[eoff] - 1),
                    )
                    seen[eoff] += 1

                win_rows = [RPW * w for w in range(NWIN)]
                for c0 in range(0, NWIN, CHUNK_WIN):
                    wins = list(range(c0, min(c0 + CHUNK_WIN, NWIN)))
                    chunk_r0 = RPW * c0
                    chunk_nr = min(RPW * len(wins), H - chunk_r0)
                    stage = stage_pool.tile(
                        [COUT, RPW * CHUNK_WIN, W], f32, tag="stage", name=f"st{b}_{c0}"
                    )

                    for g0 in range(0, len(wins), GROUP_WIN):
                        group = wins[g0 : g0 + GROUP_WIN]
                        ps = {}
                        for w in group:
                            ps[w] = psum_pool.tile(
                                [128, 512], f32, tag="ps", name=f"ps{b}_{w}"
                            )
                        if mm_order == "t_outer":
                            mm_seq = [(t, w) for t in range(taps) for w in group]
                        else:
                            mm_seq = [(t, w) for w in group for t in range(taps)]
                        for t, w in mm_seq:
                            dh, dw = divmod(t, 3)
                            s = w % D
                            r0 = win_rows[w]
                            nrows = min(RPW, H - r0)
                            N = nrows * W
                            q0 = IMG0 + (r0 + dh - 1) * W + (dw - 1)
                            nc.tensor.matmul(
                                ps[w][:, :N],
                                wT[s * K_used : (s + 1) * K_used, t, :],
                                xb[s * K_used : (s + 1) * K_used, q0 : q0 + N],
                                start=(t == 0),
                                stop=(t == taps - 1),
                            )
                        # copy-out with fused bias
                        for w in group:
                            r0 = win_rows[w]
                            nrows = min(RPW, H - r0)
                            ps_v = ps[w][:, : nrows * W].rearrange(
                                "p (r s) -> p r s", s=W
                            )
                            nc.scalar.activation(
                                out=stage[:, r0 - chunk_r0 : r0 - chunk_r0 + nrows, :],
                                in_=ps_v,
                                func=mybir.ActivationFunctionType.Identity,
                                bias=bias_sb,
                                scale=1.0,
                            )
                    # overwrite the two corrupted edge columns
                    nc.scalar.activation(
                        out=stage[:, :chunk_nr, 0:1],
                        in_=edge_ps[:, chunk_r0 : chunk_r0 + chunk_nr].unsqueeze(2),
                        func=mybir.ActivationFunctionType.Identity,
                        bias=bias_sb,
                        scale=1.0,
                    )
                    nc.scalar.activation(
                        out=stage[:, :chunk_nr, W - 1 : W],
                        in_=edge_ps[
                            :, 128 + chunk_r0 : 128 + chunk_r0 + chunk_nr
                        ].unsqueeze(2),
                        func=mybir.ActivationFunctionType.Identity,
                        bias=bias_sb,
                        scale=1.0,
                    )

                    nc.sync.dma_start(
                        out=y_out[b][:, chunk_r0 : chunk_r0 + chunk_nr, :],
                        in_=stage[:, :chunk_nr, :],
                    )

            if repeat > 1:
                loop_cm.__exit__(None, None, None)

    nc.compile()
    return nc


def kernel(x, weight, bias, mask):
    x = np.ascontiguousarray(np.asarray(x, dtype=np.float32))
    weight = np.ascontiguousarray(np.asarray(weight, dtype=np.float32))
    bias = np.ascontiguousarray(np.asarray(bias, dtype=np.float32))
    mask = np.ascontiguousarray(np.asarray(mask, dtype=np.float32))

    # --- host-side schedule specialization from the runtime mask ----------
    wm = weight * mask
    blk_any = (
        np.abs(wm).reshape(COUT, CIN // BLK, BLK, KH, KW).sum(axis=(0, 2, 3, 4)) > 0
    )
    used_ibs = [ib for ib in range(CIN // BLK) if blk_any[ib]]
    if not used_ibs:
        used_ibs = [0]
    n_ib = len(used_ibs)
    K_used = BLK * n_ib
    D = 128 // K_used if K_used <= 64 else 1

    used_ch = np.concatenate(
        [np.arange(ib * BLK, (ib + 1) * BLK) for ib in used_ibs]
    )

    key = (n_ib, D)
    if key not in _cache:
        _cache[key] = _build(n_ib, D)
    nc = _cache[key]

    w_slice = np.ascontiguousarray(weight[:, used_ch])
    m_slice = np.ascontiguousarray(mask[:, used_ch])
    rep_ch = np.concatenate([used_ch] * D)  # host-side replica duplication
    in_maps = []
    for core in range(NCORES):
        xs = np.ascontiguousarray(x[core * BPC : (core + 1) * BPC][:, rep_ch])
        in_maps.append({"x": xs, "w": w_slice, "m": m_slice, "bias": bias})

    global _last_in_maps
    _last_in_maps = in_maps

    res = bass_utils.run_bass_kernel_spmd(nc, in_maps, core_ids=list(range(NCORES)))
    y = np.concatenate([res.results[c]["y"] for c in range(NCORES)], axis=0)
    return y


_last_in_maps = None



# revision 2
# speedup vs baseline: 1.5769x; 1.5769x over previous
"""Blocksparse conv2d (3x3, stride 1, pad 1) on 8 Trainium2 NeuronCores.

Strategy
--------
Data-parallel over batch: 16 images -> 2 per core, identical SPMD program.

The mask zeroes whole 32x32 (cout, cin) channel blocks; the host inspects
the runtime mask and specializes the schedule: only surviving input-channel
blocks are shipped/loaded (K_used channels).  When 2*K_used <= 128 the two
images of a core are PAIRED across PE row groups (img0 on partitions
0:K_used, img1 on K_used:2*K_used) so their matmul streams run concurrently
on independent row tiles of the systolic array -- full-array throughput
without duplicating any x data (the previous replication scheme doubled the
x HBM traffic for the same PE rate).

Conv is lowered to 9 shifted matmuls accumulating in PSUM.  The host
pre-pads each image with a zero border (130x130), so every tap is a clean
2D-strided view of one flat SBUF buffer -- no edge-column fixup matmuls at
all.  The x load is chunked into 8 row-band DMAs so the matmul pipeline
starts as soon as the first band lands instead of waiting ~25us for the
full load (the old kernel's single-shot load serialized the whole kernel).

Weights are premultiplied by the mask, transposed and replicated on the
host, and DMA'd once.  Everything (x and weights) is cast fp32->bf16 inside
the SWDGE load DMAs: bf16 halves LDWEIGHTS time (fast-weight-load works for
non-fp32 dtypes) and keeps conv error ~2e-3, well under the 2e-2 gate.
Bias is fused into the PSUM->SBUF copy, alternating between the scalar and
vector engines so neither becomes the straggler.
"""

import numpy as np
from contextlib import ExitStack

import concourse.bass as bass
import concourse.tile as tile
from concourse import mybir, bacc
from concourse import bass_utils

# Problem shape (hardcoded per contract)
B, CIN, COUT, H, W = 16, 128, 128, 128, 128
KH, KW = 3, 3
BLK = 32
NCORES = 8
BPC = B // NCORES            # images per core
PH, PW = H + 2, W + 2        # host zero-padded image (130 x 130)
FLAT = PH * PW

RPW = 4                      # output rows per PSUM window (N = 512 = full bank)
NWIN = H // RPW              # 32 windows
CHUNK_WIN = 8                # windows staged per output DMA (32 rows, 2 MB)
GROUP_WIN = 3                # windows per matmul group per image (6 banks live)

_cache = {}
_last_in_maps = None


def _build(n_ib, paired):
    """Build + compile the per-core SPMD program.

    n_ib:   number of surviving 32-channel input blocks (1..4)
    paired: both images share the partition dim on separate PE row groups
    """
    K_used = BLK * n_ib
    reps = 2 if paired else 1
    DK = reps * K_used
    assert DK <= 128

    nc = bacc.Bacc("TRN2", target_bir_lowering=False, debug=False)
    f32 = mybir.dt.float32
    bf16 = mybir.dt.bfloat16

    x_in = nc.dram_tensor("x", [BPC, K_used, PH, PW], f32, kind="ExternalInput").ap()
    w_in = nc.dram_tensor("wt", [DK, KH * KW, COUT], f32, kind="ExternalInput").ap()
    b_in = nc.dram_tensor("bias", [COUT], f32, kind="ExternalInput").ap()
    y_out = nc.dram_tensor("y", [BPC, COUT, H, W], f32, kind="ExternalOutput").ap()

    # x-load chunk boundaries (padded-image rows): first band 18 rows, then 16
    # each -- band k fully covers the halo rows windows 4k..4k+3 read.
    bounds = [0, 18]
    while bounds[-1] < PH:
        bounds.append(min(bounds[-1] + 16, PH))

    with tile.TileContext(nc) as tc:
        with ExitStack() as ctx:
            singles = ctx.enter_context(tc.tile_pool(name="singles", bufs=1))
            stage_pool = ctx.enter_context(tc.tile_pool(name="ystage", bufs=4))
            psum_pool = ctx.enter_context(
                tc.tile_pool(name="psum", bufs=8, space="PSUM")
            )

            bias_sb = singles.tile([COUT, 1], f32, name="bias_sb")
            nc.sync.dma_start(out=bias_sb, in_=b_in.unsqueeze(1))

            # host-prepared wT[rep*K_used + c, t, o] = (w*m)[o, c, tap t]
            wT = singles.tile([DK, KH * KW, COUT], bf16, name="wT")
            nc.gpsimd.dma_start(out=wT, in_=w_in)  # fp32 -> bf16 in SWDGE

            if paired:
                xb = singles.tile([DK, FLAT], bf16, name="xb")
                src = x_in.rearrange("b c h w -> (b c) (h w)")
                for k in range(len(bounds) - 1):
                    lo, hi = bounds[k] * PW, bounds[k + 1] * PW
                    nc.gpsimd.dma_start(out=xb[:, lo:hi], in_=src[:, lo:hi])
                xbufs = [xb] * BPC
                img_base = [i * K_used for i in range(BPC)]
            else:
                xbufs, img_base = [], []
                for b in range(BPC):
                    xbi = singles.tile([K_used, FLAT], bf16, name=f"xb{b}")
                    src = x_in[b].rearrange("c h w -> c (h w)")
                    for k in range(len(bounds) - 1):
                        lo, hi = bounds[k] * PW, bounds[k + 1] * PW
                        nc.gpsimd.dma_start(out=xbi[:, lo:hi], in_=src[:, lo:hi])
                    xbufs.append(xbi)
                    img_base.append(0)

            for c0 in range(0, NWIN, CHUNK_WIN):
                wins = list(range(c0, min(c0 + CHUNK_WIN, NWIN)))
                chunk_r0 = RPW * c0
                chunk_nr = RPW * len(wins)
                stages = [
                    stage_pool.tile(
                        [COUT, RPW * CHUNK_WIN, W], f32, tag="stage",
                        name=f"st{b}_{c0}",
                    )
                    for b in range(BPC)
                ]
                for g0 in range(0, len(wins), GROUP_WIN):
                    group = wins[g0 : g0 + GROUP_WIN]
                    ps = {}
                    for b in range(BPC):
                        for w in group:
                            ps[(b, w)] = psum_pool.tile(
                                [128, 512], f32, tag="ps", name=f"ps{b}_{w}"
                            )
                    # tap-outer, image-inner: the two images' row groups
                    # alternate so each group's LDWEIGHTS hides behind the
                    # other group's matmuls.
                    for t in range(KH * KW):
                        dh, dw = divmod(t, KW)
                        for b in range(BPC):
                            base = img_base[b]
                            xbi = xbufs[b]
                            lhsT = wT[base : base + K_used, t, :]
                            for w in group:
                                q0 = (RPW * w + dh) * PW + dw
                                v = xbi[base : base + K_used, q0 : q0 + 1]
                                rhs = bass.AP(
                                    tensor=v.tensor,
                                    offset=v.offset,
                                    ap=[list(v.ap[0]), [PW, RPW], [1, W]],
                                )
                                nc.tensor.matmul(
                                    ps[(b, w)][:, : RPW * W],
                                    lhsT,
                                    rhs,
                                    start=(t == 0),
                                    stop=(t == KH * KW - 1),
                                )
                    # copy-out with fused bias, split across ACT and DVE
                    for b in range(BPC):
                        for w in group:
                            r0 = RPW * w
                            ps_v = ps[(b, w)][:, : RPW * W].rearrange(
                                "p (r s) -> p r s", s=W
                            )
                            dst = stages[b][:, r0 - chunk_r0 : r0 - chunk_r0 + RPW, :]
                            if (w + b) % 2 == 0:
                                nc.scalar.activation(
                                    out=dst,
                                    in_=ps_v,
                                    func=mybir.ActivationFunctionType.Identity,
                                    bias=bias_sb,
                                    scale=1.0,
                                )
                            else:
                                nc.vector.tensor_scalar_add(
                                    out=dst, in0=ps_v, scalar1=bias_sb
                                )
                for b in range(BPC):
                    nc.sync.dma_start(
                        out=y_out[b][:, chunk_r0 : chunk_r0 + chunk_nr, :],
                        in_=stages[b][:, :chunk_nr, :],
                    )

    nc.compile()
    return nc


def kernel(x, weight, bias, mask):
    x = np.ascontiguousarray(np.asarray(x, dtype=np.float32))
    weight = np.asarray(weight, dtype=np.float32)
    bias = np.ascontiguousarray(np.asarray(bias, dtype=np.float32))
    mask = np.asarray(mask, dtype=np.float32)

    # --- host-side schedule specialization from the runtime mask ----------
    wm = weight * mask
    blk_any = (
        np.abs(wm).reshape(COUT, CIN // BLK, BLK, KH, KW).sum(axis=(0, 2, 3, 4)) > 0
    )
    used_ibs = [ib for ib in range(CIN // BLK) if blk_any[ib]] or [0]
    n_ib = len(used_ibs)
    K_used = BLK * n_ib
    paired = (BPC == 2) and (2 * K_used <= 128)
    reps = 2 if paired else 1

    used_ch = np.concatenate(
        [np.arange(ib * BLK, (ib + 1) * BLK) for ib in used_ibs]
    )

    key = (n_ib, paired)
    if key not in _cache:
        _cache[key] = _build(n_ib, paired)
    nc = _cache[key]

    # wT[rep*K_used + c, t, o] = (w*m)[o, used_ch[c], tap t]
    wT = wm[:, used_ch].reshape(COUT, K_used, KH * KW).transpose(1, 2, 0)
    wT = np.ascontiguousarray(
        np.concatenate([wT] * reps, axis=0), dtype=np.float32
    )

    # zero-padded x (130x130) restricted to the used channels
    xp = np.zeros((B, K_used, PH, PW), dtype=np.float32)
    xp[:, :, 1 : H + 1, 1 : W + 1] = x[:, used_ch]

    in_maps = []
    for core in range(NCORES):
        xs = np.ascontiguousarray(xp[core * BPC : (core + 1) * BPC])
        in_maps.append({"x": xs, "wt": wT, "bias": bias})

    global _last_in_maps
    _last_in_maps = in_maps

    res = bass_utils.run_bass_kernel_spmd(nc, in_maps, core_ids=list(range(NCORES)))
    y = np.concatenate([res.results[c]["y"] for c in range(NCORES)], axis=0)
    return y


# revision 6
# speedup vs baseline: 1.6968x; 1.0760x over previous
"""Blocksparse conv2d (3x3, stride 1, pad 1) on 8 Trainium2 NeuronCores.

Strategy
--------
Data-parallel over batch: 16 images -> 2 per core, identical SPMD program.

The mask zeroes whole 32x32 (cout, cin) channel blocks; the host inspects
the runtime mask and specializes the schedule: only surviving input-channel
blocks are shipped/loaded (K_used channels).  When 2*K_used <= 128 the two
images of a core are PAIRED across PE row groups (img0 on partitions
0:K_used, img1 on K_used:2*K_used) so their matmul streams run concurrently
on independent row tiles of the systolic array -- full-array throughput
without duplicating any x data (the previous replication scheme doubled the
x HBM traffic for the same PE rate).

Conv is lowered to 9 shifted matmuls accumulating in PSUM.  The host
pre-pads each image with a zero border (130x130), so every tap is a clean
2D-strided view of one flat SBUF buffer -- no edge-column fixup matmuls at
all.  The x load is chunked into 8 row-band DMAs so the matmul pipeline
starts as soon as the first band lands instead of waiting ~25us for the
full load (the old kernel's single-shot load serialized the whole kernel).

Weights are premultiplied by the mask, transposed and replicated on the
host, and DMA'd once.  Everything (x and weights) is cast fp32->bf16 inside
the SWDGE load DMAs: bf16 halves LDWEIGHTS time (fast-weight-load works for
non-fp32 dtypes) and keeps conv error ~2e-3, well under the 2e-2 gate.
Bias is fused into the PSUM->SBUF copy, alternating between the scalar and
vector engines so neither becomes the straggler.
"""

import numpy as np
from contextlib import ExitStack

import concourse.bass as bass
import concourse.tile as tile
from concourse import mybir, bacc
from concourse import bass_utils

# Problem shape (hardcoded per contract)
B, CIN, COUT, H, W = 16, 128, 128, 128, 128
KH, KW = 3, 3
BLK = 32
NCORES = 8
BPC = B // NCORES            # images per core
PH, PW = H + 2, W + 2        # host zero-padded image (130 x 130)
FLAT = PH * PW

RPW = 4                      # output rows per PSUM window (N = 512 = full bank)
NWIN = H // RPW              # 32 windows
CHUNKS = [8, 8, 8, 4, 2, 2]  # windows per output-DMA chunk (tapered tail so the
                             # final y store drains in ~2us instead of ~12us)
GROUP_WIN = 3                # windows per matmul group per image (6 banks live)

_cache = {}
_last_in_maps = None


def _build(n_ib, paired):
    """Build + compile the per-core SPMD program.

    n_ib:   number of surviving 32-channel input blocks (1..4)
    paired: both images share the partition dim on separate PE row groups
    """
    K_used = BLK * n_ib
    reps = 2 if paired else 1
    DK = reps * K_used
    assert DK <= 128

    nc = bacc.Bacc("TRN2", target_bir_lowering=False, debug=False)
    f32 = mybir.dt.float32
    bf16 = mybir.dt.bfloat16

    x_in = nc.dram_tensor("x", [BPC, K_used, PH, PW], f32, kind="ExternalInput").ap()
    w_in = nc.dram_tensor("wt", [DK, KH * KW, COUT], f32, kind="ExternalInput").ap()
    b_in = nc.dram_tensor("bias", [COUT], f32, kind="ExternalInput").ap()
    y_out = nc.dram_tensor("y", [BPC, COUT, H, W], f32, kind="ExternalOutput").ap()

    # x-load chunk boundaries (padded-image rows).  Window w reads padded rows
    # 4w..4w+5, so a boundary at 4k+6 releases windows 0..k.  Fine-grained
    # early chunks let the matmul pipeline start ~3us in; coarser later ones
    # keep the DMA count (and SWDGE descriptor overhead) low.
    bounds = [0, 6, 14, 22, 30]
    while bounds[-1] + 16 < PH:
        bounds.append(bounds[-1] + 16)
    bounds.append(PH)

    with tile.TileContext(nc) as tc:
        with ExitStack() as ctx:
            singles = ctx.enter_context(tc.tile_pool(name="singles", bufs=1))
            stage_pool = ctx.enter_context(tc.tile_pool(name="ystage", bufs=4))
            psum_pool = ctx.enter_context(
                tc.tile_pool(name="psum", bufs=8, space="PSUM")
            )

            bias_sb = singles.tile([COUT, 1], f32, name="bias_sb")
            nc.sync.dma_start(out=bias_sb, in_=b_in.unsqueeze(1))

            # host-prepared wT[rep*K_used + c, t, o] = (w*m)[o, c, tap t].
            # HWDGE load + DVE cast keeps it off the SWDGE queue, so the x
            # chunk DMAs (which gate the matmul start) go first on gpsimd.
            wT_f32 = singles.tile([DK, KH * KW, COUT], f32, name="wT_f32")
            wT = singles.tile([DK, KH * KW, COUT], bf16, name="wT")
            nc.sync.dma_start(out=wT_f32, in_=w_in)
            nc.vector.tensor_copy(out=wT, in_=wT_f32)

            if paired:
                xb = singles.tile([DK, FLAT], bf16, name="xb")
                src = x_in.rearrange("b c h w -> (b c) (h w)")
                for k in range(len(bounds) - 1):
                    lo, hi = bounds[k] * PW, bounds[k + 1] * PW
                    nc.gpsimd.dma_start(out=xb[:, lo:hi], in_=src[:, lo:hi])
                xbufs = [xb] * BPC
                img_base = [i * K_used for i in range(BPC)]
            else:
                xbufs, img_base = [], []
                for b in range(BPC):
                    xbi = singles.tile([K_used, FLAT], bf16, name=f"xb{b}")
                    src = x_in[b].rearrange("c h w -> c (h w)")
                    for k in range(len(bounds) - 1):
                        lo, hi = bounds[k] * PW, bounds[k + 1] * PW
                        nc.gpsimd.dma_start(out=xbi[:, lo:hi], in_=src[:, lo:hi])
                    xbufs.append(xbi)
                    img_base.append(0)

            assert sum(CHUNKS) == NWIN
            c0 = 0
            for nwc in CHUNKS:
                wins = list(range(c0, c0 + nwc))
                chunk_r0 = RPW * c0
                chunk_nr = RPW * len(wins)
                c0 += nwc
                stages = [
                    stage_pool.tile(
                        [COUT, RPW * max(CHUNKS), W], f32, tag="stage",
                        name=f"st{b}_{chunk_r0}",
                    )
                    for b in range(BPC)
                ]
                for g0 in range(0, len(wins), GROUP_WIN):
                    group = wins[g0 : g0 + GROUP_WIN]
                    ps = {}
                    for b in range(BPC):
                        for w in group:
                            ps[(b, w)] = psum_pool.tile(
                                [128, 512], f32, tag="ps", name=f"ps{b}_{w}"
                            )
                    # tap-outer, image-inner: the two images' row groups
                    # alternate so each group's LDWEIGHTS hides behind the
                    # other group's matmuls.
                    for t in range(KH * KW):
                        dh, dw = divmod(t, KW)
                        for b in range(BPC):
                            base = img_base[b]
                            xbi = xbufs[b]
                            lhsT = wT[base : base + K_used, t, :]
                            for w in group:
                                q0 = (RPW * w + dh) * PW + dw
                                v = xbi[base : base + K_used, q0 : q0 + 1]
                                rhs = bass.AP(
                                    tensor=v.tensor,
                                    offset=v.offset,
                                    ap=[list(v.ap[0]), [PW, RPW], [1, W]],
                                )
                                nc.tensor.matmul(
                                    ps[(b, w)][:, : RPW * W],
                                    lhsT,
                                    rhs,
                                    start=(t == 0),
                                    stop=(t == KH * KW - 1),
                                )
                    # copy-out with fused bias, split across ACT and DVE
                    for b in range(BPC):
                        for w in group:
                            r0 = RPW * w
                            ps_v = ps[(b, w)][:, : RPW * W].rearrange(
                                "p (r s) -> p r s", s=W
                            )
                            dst = stages[b][:, r0 - chunk_r0 : r0 - chunk_r0 + RPW, :]
                            if (w + b) % 2 == 0:
                                nc.scalar.activation(
                                    out=dst,
                                    in_=ps_v,
                                    func=mybir.ActivationFunctionType.Identity,
                                    bias=bias_sb,
                                    scale=1.0,
                                )
                            else:
                                nc.vector.tensor_scalar_add(
                                    out=dst, in0=ps_v, scalar1=bias_sb
                                )
                for b in range(BPC):
                    nc.sync.dma_start(
                        out=y_out[b][:, chunk_r0 : chunk_r0 + chunk_nr, :],
                        in_=stages[b][:, :chunk_nr, :],
                    )

    nc.compile()
    return nc


def kernel(x, weight, bias, mask):
    x = np.ascontiguousarray(np.asarray(x, dtype=np.float32))
    weight = np.asarray(weight, dtype=np.float32)
    bias = np.ascontiguousarray(np.asarray(bias, dtype=np.float32))
    mask = np.asarray(mask, dtype=np.float32)

    # --- host-side schedule specialization from the runtime mask ----------
    wm = weight * mask
    blk_any = (
        np.abs(wm).reshape(COUT, CIN // BLK, BLK, KH, KW).sum(axis=(0, 2, 3, 4)) > 0
    )
    used_ibs = [ib for ib in range(CIN // BLK) if blk_any[ib]] or [0]
    n_ib = len(used_ibs)
    K_used = BLK * n_ib
    paired = (BPC == 2) and (2 * K_used <= 128)
    reps = 2 if paired else 1

    used_ch = np.concatenate(
        [np.arange(ib * BLK, (ib + 1) * BLK) for ib in used_ibs]
    )

    key = (n_ib, paired)
    if key not in _cache:
        _cache[key] = _build(n_ib, paired)
    nc = _cache[key]

    # wT[rep*K_used + c, t, o] = (w*m)[o, used_ch[c], tap t]
    wT = wm[:, used_ch].reshape(COUT, K_used, KH * KW).transpose(1, 2, 0)
    wT = np.ascontiguousarray(
        np.concatenate([wT] * reps, axis=0), dtype=np.float32
    )

    # zero-padded x (130x130) restricted to the used channels
    xp = np.zeros((B, K_used, PH, PW), dtype=np.float32)
    xp[:, :, 1 : H + 1, 1 : W + 1] = x[:, used_ch]

    in_maps = []
    for core in range(NCORES):
        xs = np.ascontiguousarray(xp[core * BPC : (core + 1) * BPC])
        in_maps.append({"x": xs, "wt": wT, "bias": bias})

    global _last_in_maps
    _last_in_maps = in_maps

    res = bass_utils.run_bass_kernel_spmd(nc, in_maps, core_ids=list(range(NCORES)))
    y = np.concatenate([res.results[c]["y"] for c in range(NCORES)], axis=0)
    return y
